# revision 9
# baseline (speedup 1.0000x reference)
"""Trainium2 Bass kernel for nn_FDGRPretrainedModel (loss_fn).

Self-contained: accepts FULL inputs, shards across 8 NeuronCores internally,
returns (losses[5], ha[8,128,256], hc[8,128,256]).

Sharding: data-parallel over the 8 batch rows for encoders/decoder/vad/orth;
the [512,512] InfoNCE score matrix is column-sharded (64 cols/core) with an
AllGather of bf16 ha_T chunks in between; per-core partial sumexp vectors are
combined on host.
"""
import numpy as np
import ml_dtypes

import concourse.bass as bass
import concourse.mybir as mybir
import concourse.tile as tile
from concourse import bacc
from concourse import bass_utils
from concourse.masks import make_identity

B, S, H, HD = 4, 128, 768, 256
NCORES = 8
N = B * S            # 512 InfoNCE rows/cols
JSH = N // NCORES    # 64 T1 columns per core
P = 128
EPS = 1e-12
PEN = -30000.0       # mask penalty (exp underflows to exactly 0)

F32 = mybir.dt.float32
F32R = mybir.dt.float32r
BF16 = mybir.dt.bfloat16
AF = mybir.ActivationFunctionType
ALU = mybir.AluOpType
BF16NP = ml_dtypes.bfloat16


def _f32(x):
    return np.ascontiguousarray(np.asarray(x, dtype=np.float32))


def _bf16(x):
    return np.ascontiguousarray(np.asarray(x, dtype=np.float32).astype(BF16NP))


_COMPILED = None


def _build():
    nc = bacc.Bacc("TRN2", target_bir_lowering=False, debug=False,
                   enable_asserts=True, num_devices=NCORES)

    def din(name, shape, dt):
        return nc.dram_tensor(name, shape, dt, kind="ExternalInput").ap()

    def dout(name, shape, dt):
        return nc.dram_tensor(name, shape, dt, kind="ExternalOutput").ap()

    # ---- inputs (per-core variation only via values, never addresses) ----
    xT = din("xT", [H, P], F32R)          # seq_output[batch].T
    xmbt = din("xmbt", [P, H], F32)       # seq_output[batch] - de_bt
    vad_t = din("vad_t", [P, 3], F32)     # vad target for this batch
    pen = din("pen", [JSH, N], BF16)      # penalty rows for this j-shard
    selm = din("selm", [N, JSH], BF16)    # one-hot j-shard selector
    eyepad = din("eyepad", [JSH, JSH + 32], BF16)

    w_ha1 = din("w_ha1", [H, H], F32R)
    w_ha2 = din("w_ha2", [H, HD], F32R)
    w_hc1 = din("w_hc1", [H, H], F32R)
    w_hc2 = din("w_hc2", [H, HD], F32R)
    w_de1 = din("w_de1", [2 * HD, H], F32R)
    w_de2 = din("w_de2", [H, H], F32R)
    w_v = din("w_v", [HD, HD], F32R)
    w_a = din("w_a", [HD, HD], F32R)
    w_d = din("w_d", [HD, HD], F32R)
    w_proj = din("w_proj", [HD, 4], F32R)
    fw1 = din("fw1", [2 * HD, HD], BF16)
    w2ext = din("w2ext", [HD, 32], BF16)  # f_w2 in col 0, zeros elsewhere
    selv = din("selv", [P, 1], BF16)      # 1.0 at partitions 0,32,64,96

    b_ha1 = din("b_ha1", [1, H], F32R)
    b_ha2 = din("b_ha2", [1, HD], F32R)
    b_hc1 = din("b_hc1", [1, H], F32R)
    b_hc2 = din("b_hc2", [1, HD], F32R)
    b_de1 = din("b_de1", [1, H], F32R)
    b_de2 = din("b_de2", [1, H], F32R)
    b_v = din("b_v", [1, HD], F32R)
    b_a = din("b_a", [1, HD], F32R)
    b_d = din("b_d", [1, HD], F32R)
    b_proj = din("b_proj", [1, 4], F32R)
    fb1 = din("fb1", [HD, 1], F32)
    fb2 = din("fb2", [1, 1], F32)
    ones_r = din("ones_r", [1, P], F32R)

    g_ha = din("g_ha", [1, HD], F32)
    t_ha = din("t_ha", [1, HD], F32)
    g_hc = din("g_hc", [1, HD], F32)
    t_hc = din("t_hc", [1, HD], F32)
    g_de = din("g_de", [1, H], F32)

    ha_out = dout("ha_out", [P, HD], F32)
    hc_out = dout("hc_out", [P, HD], F32)
    partial = dout("partial", [1, N], F32)
    scal = dout("scal", [1, 8], F32)

    X = mybir.AxisListType.X

    with tile.TileContext(nc) as tc:
        wp = tc.alloc_tile_pool(name="wp", bufs=1)
        wbig = tc.alloc_tile_pool(name="wbig", bufs=2)
        ap_ = tc.alloc_tile_pool(name="ap", bufs=1)
        dp = tc.alloc_tile_pool(name="dp", bufs=1, space="DRAM")

        # ---- broadcast loads (gpsimd; issued before the collective) ----
        def bcast(name, src, width):
            t = wp.tile([P, width], F32, tag=name)
            bc = bass.AP(tensor=src.tensor, offset=src.offset,
                         ap=[[0, P]] + src.ap[1:])
            nc.gpsimd.dma_start(out=t, in_=bc)
            return t

        gha_bc = bcast("gha_bc", g_ha, HD)
        tha_bc = bcast("tha_bc", t_ha, HD)
        ghc_bc = bcast("ghc_bc", g_hc, HD)
        thc_bc = bcast("thc_bc", t_hc, HD)
        gde_bc = bcast("gde_bc", g_de, H)
        fb2_bc = bcast("fb2_bc", fb2, 1)

        # ---- plain loads; [F, C] with F>128 lands as [128, F//128, C] ----
        def load(name, src, shape, dt, pool=None):
            pool = pool or wp
            if shape[0] > P:
                kt = shape[0] // P
                t = pool.tile([P, kt, shape[1]], dt, tag=name)
                nc.sync.dma_start(
                    out=t, in_=src.rearrange("(k p) c -> p k c", p=P))
            else:
                t = pool.tile(shape, dt, tag=name)
                nc.sync.dma_start(out=t, in_=src)
            return t

        xT_t = load("xT_t", xT, [H, P], F32R)
        xmbt_t = load("xmbt_t", xmbt, [P, H], F32)
        vad_tt = load("vad_tt", vad_t, [P, 3], F32)
        pen_t = load("pen_t", pen, [JSH, N], BF16)
        selm_t = load("selm_t", selm, [N, JSH], BF16)
        eyep_t = load("eyep_t", eyepad, [JSH, JSH + 32], BF16)
        wha1_t = load("wbig", w_ha1, [H, H], F32R, pool=wbig)
        wha2_t = load("wha2_t", w_ha2, [H, HD], F32R)
        whc1_t = load("wbig", w_hc1, [H, H], F32R, pool=wbig)
        whc2_t = load("whc2_t", w_hc2, [H, HD], F32R)
        wde1_t = load("wde1_t", w_de1, [2 * HD, H], F32R)
        wde2_t = load("wbig", w_de2, [H, H], F32R, pool=wbig)
        wv_t = load("wv_t", w_v, [HD, HD], F32R)
        wa_t = load("wa_t", w_a, [HD, HD], F32R)
        wd_t = load("wd_t", w_d, [HD, HD], F32R)
        wproj_t = load("wproj_t", w_proj, [HD, 4], F32R)
        fw1_t = load("fw1_t", fw1, [2 * HD, HD], BF16)
        w2ext_t = load("w2ext_t", w2ext, [HD, 32], BF16)
        selv_t = load("selv_t", selv, [P, 1], BF16)
        bha1_t = load("bha1_t", b_ha1, [1, H], F32R)
        bha2_t = load("bha2_t", b_ha2, [1, HD], F32R)
        bhc1_t = load("bhc1_t", b_hc1, [1, H], F32R)
        bhc2_t = load("bhc2_t", b_hc2, [1, HD], F32R)
        bde1_t = load("bde1_t", b_de1, [1, H], F32R)
        bde2_t = load("bde2_t", b_de2, [1, H], F32R)
        bv_t = load("bv_t", b_v, [1, HD], F32R)
        ba_t = load("ba_t", b_a, [1, HD], F32R)
        bd_t = load("bd_t", b_d, [1, HD], F32R)
        bproj_t = load("bproj_t", b_proj, [1, 4], F32R)
        fb1_t = load("fb1_t", fb1.rearrange("(k p) c -> p (k c)", p=P),
                     [P, 2], F32)
        ones_t = load("ones_t", ones_r, [1, P], F32R)

        id_f = wp.tile([P, P], F32)
        make_identity(nc, id_f)
        id_r = wp.tile([P, P], F32R)
        nc.vector.tensor_copy(id_r, id_f)
        eps_t = wp.tile([P, 1], F32)
        nc.vector.memset(eps_t, EPS)
        ones_col = wp.tile([P, 1], F32)
        nc.vector.memset(ones_col, 1.0)

        ag_in = dp.tile([2 * P, P], BF16)
        ag_out = dp.tile([NCORES * 2 * P, P], BF16, addr_space="Shared")

        CH = H // 2
        scal_sb = ap_.tile([1, 8], F32)
        nc.vector.memset(scal_sb, 0.0)

        pp = tc.alloc_tile_pool(name="pp", bufs=4, space="PSUM")

        # ============ encoder (row-major, fp32r) ============
        def encoder(w1_t, b1_t, w2_t, b2_t, g_bc, t_bc, tagp):
            h1 = ap_.tile([P, H], F32R, tag=tagp + "h1")
            for ch in range(2):
                ps1 = pp.tile([P, CH], F32, tag="ps")
                for k in range(6):
                    nc.tensor.matmul(
                        ps1, xT_t[:, k, :],
                        w1_t[:, k, ch * CH:(ch + 1) * CH],
                        start=(k == 0), stop=False)
                nc.tensor.matmul(ps1, ones_t, b1_t[0:1, ch * CH:(ch + 1) * CH],
                                 start=False, stop=True)
                nc.scalar.activation(h1[:, ch * CH:(ch + 1) * CH], ps1, AF.Relu)
            h1T = ap_.tile([P, 6, P], F32R, tag=tagp + "h1T")
            for k in range(6):
                pst = pp.tile([P, P], F32R, tag="ps")
                nc.tensor.transpose(pst, h1[:, k * P:(k + 1) * P], id_r)
                nc.vector.tensor_copy(h1T[:, k, :], pst)
            ps2 = pp.tile([P, HD], F32, tag="ps")
            for k in range(6):
                nc.tensor.matmul(ps2, h1T[:, k, :], w2_t[:, k, :],
                                 start=(k == 0), stop=False)
            nc.tensor.matmul(ps2, ones_t, b2_t, start=False, stop=True)
            h2 = ap_.tile([P, HD], F32, tag=tagp + "h2")
            nc.scalar.activation(h2, ps2, AF.Relu)
            stats = ap_.tile([P, nc.vector.BN_STATS_DIM], F32, tag=tagp + "st")
            nc.vector.bn_stats(out=stats, in_=h2)
            mv = ap_.tile([P, nc.vector.BN_AGGR_DIM], F32, tag=tagp + "mv")
            nc.vector.bn_aggr(out=mv, in_=stats)
            negm = ap_.tile([P, 1], F32, tag=tagp + "negm")
            nc.vector.tensor_scalar_mul(negm, mv[:, 0:1], -1.0)
            sd = ap_.tile([P, 1], F32, tag=tagp + "sd")
            nc.scalar.activation(sd, mv[:, 1:2], AF.Sqrt, bias=eps_t)
            rstd = ap_.tile([P, 1], F32, tag=tagp + "rstd")
            nc.vector.reciprocal(out=rstd, in_=sd)
            fin = ap_.tile([P, HD], F32, tag=tagp + "fin")
            nc.vector.tensor_scalar(fin, h2, negm, rstd, ALU.add, ALU.mult)
            nc.vector.tensor_mul(fin, fin, g_bc[:, 0:HD])
            nc.vector.tensor_add(fin, fin, t_bc[:, 0:HD])
            finT = ap_.tile([P, 2, P], F32R, tag=tagp + "finT")
            for k in range(2):
                pst2 = pp.tile([P, P], F32, tag="ps")
                nc.tensor.transpose(pst2, fin[:, k * P:(k + 1) * P], id_f)
                nc.vector.tensor_copy(finT[:, k, :], pst2)
            return fin, finT

        ha_fin, haT = encoder(wha1_t, bha1_t, wha2_t, bha2_t,
                              gha_bc, tha_bc, "ha")
        nc.sync.dma_start(out=ha_out, in_=ha_fin)

        haT_bf = ap_.tile([P, 2, P], BF16, tag="haT_bf")
        nc.vector.tensor_copy(haT_bf[:, 0, :], haT[:, 0, :])
        nc.vector.tensor_copy(haT_bf[:, 1, :], haT[:, 1, :])
        nc.gpsimd.dma_start(
            out=ag_in.rearrange("(k p) c -> p k c", p=P), in_=haT_bf)
        nc.gpsimd.collective_compute(
            "AllGather", ALU.bypass,
            ins=[ag_in.opt()], outs=[ag_out.opt()],
            replica_groups=[list(range(NCORES))])

        hc_fin, hcT = encoder(whc1_t, bhc1_t, whc2_t, bhc2_t,
                              ghc_bc, thc_bc, "hc")
        nc.sync.dma_start(out=hc_out, in_=hc_fin)

        # ============ decoder ============
        catT = [haT[:, 0, :], haT[:, 1, :], hcT[:, 0, :], hcT[:, 1, :]]
        d1 = ap_.tile([P, H], F32R, tag="d1")
        for ch in range(2):
            psd = pp.tile([P, CH], F32, tag="ps")
            for k in range(4):
                nc.tensor.matmul(psd, catT[k],
                                 wde1_t[:, k, ch * CH:(ch + 1) * CH],
                                 start=(k == 0), stop=False)
            nc.tensor.matmul(psd, ones_t, bde1_t[0:1, ch * CH:(ch + 1) * CH],
                             start=False, stop=True)
            nc.scalar.activation(d1[:, ch * CH:(ch + 1) * CH], psd, AF.Relu)
        d1T = ap_.tile([P, 6, P], F32R, tag="d1T")
        for k in range(6):
            pst = pp.tile([P, P], F32R, tag="ps")
            nc.tensor.transpose(pst, d1[:, k * P:(k + 1) * P], id_r)
            nc.vector.tensor_copy(d1T[:, k, :], pst)
        d2 = ap_.tile([P, H], F32, tag="d2")
        for ch in range(2):
            psd2 = pp.tile([P, CH], F32, tag="ps")
            for k in range(6):
                nc.tensor.matmul(psd2, d1T[:, k, :],
                                 wde2_t[:, k, ch * CH:(ch + 1) * CH],
                                 start=(k == 0), stop=False)
            nc.tensor.matmul(psd2, ones_t, bde2_t[0:1, ch * CH:(ch + 1) * CH],
                             start=False, stop=True)
            nc.scalar.activation(d2[:, ch * CH:(ch + 1) * CH], psd2, AF.Relu)
        dstats = ap_.tile([P, 3, nc.vector.BN_STATS_DIM], F32)
        for sg in range(3):
            nc.vector.bn_stats(out=dstats[:, sg, :],
                               in_=d2[:, sg * 256:(sg + 1) * 256])
        dmv = ap_.tile([P, nc.vector.BN_AGGR_DIM], F32)
        nc.vector.bn_aggr(out=dmv, in_=dstats)
        dnegm = ap_.tile([P, 1], F32)
        nc.vector.tensor_scalar_mul(dnegm, dmv[:, 0:1], -1.0)
        dsd = ap_.tile([P, 1], F32)
        nc.scalar.activation(dsd, dmv[:, 1:2], AF.Sqrt, bias=eps_t)
        drstd = ap_.tile([P, 1], F32)
        nc.vector.reciprocal(out=drstd, in_=dsd)
        dlnc = ap_.tile([P, H], F32)
        nc.vector.tensor_scalar(dlnc, d2, dnegm, drstd, ALU.add, ALU.mult)
        # rec partial: sum((lnc*g - (x - bt))^2), in place
        nc.vector.tensor_mul(dlnc, dlnc, gde_bc)
        nc.vector.tensor_sub(dlnc, dlnc, xmbt_t)
        recacc = ap_.tile([P, 1], F32)
        nc.scalar.activation(dlnc, dlnc, AF.Square, accum_out=recacc)

        # club partial: sum (ha-hc)^2, clobbers hc_fin (already stored/T'd)
        clubacc = ap_.tile([P, 1], F32)
        nc.vector.tensor_sub(hc_fin, ha_fin, hc_fin)
        nc.scalar.activation(hc_fin, hc_fin, AF.Square, accum_out=clubacc)

        # ============ vad + orthogonal ============
        vad3 = []
        for wt, bt_, tag in ((wv_t, bv_t, "v"), (wa_t, ba_t, "a"),
                             (wd_t, bd_t, "d")):
            psv = pp.tile([P, HD], F32, tag="ps")
            for k in range(2):
                nc.tensor.matmul(psv, hcT[:, k, :], wt[:, k, :],
                                 start=(k == 0), stop=False)
            nc.tensor.matmul(psv, ones_t, bt_, start=False, stop=True)
            vt = ap_.tile([P, HD], F32, tag="vad" + tag)
            nc.vector.tensor_copy(vt, psv)
            vad3.append(vt)
        pspj = pp.tile([P, 4], F32, tag="ps")
        for k in range(2):
            nc.tensor.matmul(pspj, hcT[:, k, :], wproj_t[:, k, :],
                             start=(k == 0), stop=False)
        nc.tensor.matmul(pspj, ones_t, bproj_t, start=False, stop=True)
        pdiff = ap_.tile([P, 3], F32)
        nc.vector.tensor_sub(pdiff, pspj[:, 0:3], vad_tt)
        vadacc = ap_.tile([P, 1], F32)
        nc.scalar.activation(pdiff, pdiff, AF.Square, accum_out=vadacc)
        grams = ap_.tile([P, 6], F32)
        for i, vt in enumerate(vad3):
            sq = ap_.tile([P, HD], F32, tag="orth_sq")
            nc.scalar.activation(sq, vt, AF.Square,
                                 accum_out=grams[:, i:i + 1])
        for i, (p0, p1) in enumerate([(0, 1), (0, 2), (1, 2)]):
            pr = ap_.tile([P, HD], F32, tag="orth_pr")
            nc.vector.tensor_mul(pr, vad3[p0], vad3[p1])
            nc.vector.tensor_reduce(out=grams[:, 3 + i:4 + i], in_=pr,
                                    axis=X, op=ALU.add)
        nc.vector.tensor_scalar(grams[:, 0:3], grams[:, 0:3], -1.0, None,
                                ALU.add)
        nc.scalar.activation(grams, grams, AF.Square)
        orthacc = ap_.tile([P, 1], F32)
        nc.vector.tensor_reduce(out=orthacc, in_=grams[:, 0:3], axis=X,
                                op=ALU.add)
        orthacc2 = ap_.tile([P, 1], F32)
        nc.vector.tensor_reduce(out=orthacc2, in_=grams[:, 3:6], axis=X,
                                op=ALU.add)
        nc.vector.tensor_scalar(orthacc2, orthacc2, 2.0, None, ALU.mult)
        nc.vector.tensor_add(orthacc, orthacc, orthacc2)

        accs = ap_.tile([P, 4], F32)
        nc.vector.tensor_copy(accs[:, 0:1], clubacc)
        nc.vector.tensor_copy(accs[:, 1:2], recacc)
        nc.vector.tensor_copy(accs[:, 2:3], vadacc)
        nc.vector.tensor_copy(accs[:, 3:4], orthacc)
        ps_sc = pp.tile([1, 4], F32, tag="ps")
        nc.tensor.matmul(ps_sc, ones_col, accs, start=True, stop=True)
        nc.vector.tensor_copy(scal_sb[0:1, 0:4], ps_sc)

        # ============ AllGather readback ============
        origT = ap_.tile([P, 2, N], BF16, tag="origT")
        contT = ap_.tile([P, 2, N], BF16, tag="contT")
        for bc_ in range(4):
            for half in range(2):
                nc.sync.dma_start(
                    out=origT[:, half, bc_ * P:(bc_ + 1) * P],
                    in_=ag_out[bc_ * 2 * P + half * P:
                               bc_ * 2 * P + (half + 1) * P, :])
                nc.sync.dma_start(
                    out=contT[:, half, bc_ * P:(bc_ + 1) * P],
                    in_=ag_out[(4 + bc_) * 2 * P + half * P:
                               (4 + bc_) * 2 * P + (half + 1) * P, :])

        # C_T = W1c.T @ contT  [HD, N]
        cT = ap_.tile([P, 2, N], BF16, tag="cT")
        for m in range(2):
            psc = pp.tile([P, N], F32, tag="ps")
            for k in range(2):
                nc.tensor.matmul(psc, fw1_t[:, 2 + k, m * P:(m + 1) * P],
                                 contT[:, k, :],
                                 start=(k == 0), stop=(k == 1))
            nc.vector.tensor_copy(cT[:, m, :], psc)

        # acolb = (A_T + f_b1) restricted to the j-shard  [HD, JSH]
        arow = ap_.tile([P, 4, HD], BF16, tag="arow")
        for m in range(4):
            psa = pp.tile([P, HD], F32, tag="ps")
            for k in range(2):
                nc.tensor.matmul(psa, origT[:, k, m * P:(m + 1) * P],
                                 fw1_t[:, k, :],
                                 start=(k == 0), stop=(k == 1))
            nc.vector.tensor_copy(arow[:, m, :], psa)
        ps_ar = pp.tile([JSH, HD], F32, tag="ps")
        for k in range(4):
            nc.tensor.matmul(ps_ar, selm_t[:, k, :], arow[:, k, :],
                             start=(k == 0), stop=(k == 3))
        acol_row = ap_.tile([JSH, HD], F32, tag="acol_row")
        nc.vector.tensor_copy(acol_row, ps_ar)
        acolb = ap_.tile([P, 2, JSH], F32, tag="acolb")
        for k in range(2):
            pst3 = pp.tile([P, JSH], F32, tag="ps")
            nc.tensor.transpose(pst3, acol_row[:, k * P:(k + 1) * P], id_f[0:JSH, 0:JSH])
            nc.vector.tensor_scalar(acolb[:, k, :], pst3,
                                    fb1_t[:, k:k + 1], None, ALU.add)

        # T0 over all 512 rows (identical on every core; host uses core 0)
        x0T = [origT[:, 0, :], origT[:, 1, :], contT[:, 0, :], contT[:, 1, :]]
        h0 = ap_.tile([P, 2, N], BF16, tag="h0")
        for m in range(2):
            psh = pp.tile([P, N], F32, tag="ps")
            for k in range(4):
                nc.tensor.matmul(psh, fw1_t[:, k, m * P:(m + 1) * P],
                                 x0T[k], start=(k == 0), stop=(k == 3))
            nc.scalar.activation(h0[:, m, :], psh, AF.Relu,
                                 bias=fb1_t[:, m:m + 1])
        ps_z0 = pp.tile([32, N], F32, tag="ps")
        for k in range(2):
            nc.tensor.matmul(ps_z0, w2ext_t[:, k, :], h0[:, k, :],
                             start=(k == 0), stop=(k == 1))
        e0 = ap_.tile([32, N], F32, tag="e0")
        nc.scalar.activation(e0, ps_z0, AF.Exp, bias=fb2_bc[0:32, 0:1])
        lg = ap_.tile([1, N], F32, tag="lg")
        nc.vector.tensor_scalar(lg, e0[0:1, :], 1.0, None, ALU.add)
        t0acc = ap_.tile([1, 1], F32, tag="t0acc")
        nc.scalar.activation(lg, lg, AF.Ln, accum_out=t0acc)
        nc.vector.tensor_copy(scal_sb[0:1, 4:5], t0acc)
        nc.sync.dma_start(out=scal, in_=scal_sb)

        pp.release()

        # ============ T1 main loop ============
        zpp = tc.alloc_tile_pool(name="zpp", bufs=2, space="PSUM")
        rp = tc.alloc_tile_pool(name="rp", bufs=8)
        E = ap_.tile([P, 4 * 2048], BF16, tag="E")
        for g in range(4):
            zp = zpp.tile([P, 2048], F32, tag="zp")
            for b_ in range(4):
                for cg in range(4):
                    jl = g * 16 + b_ * 4 + cg
                    outap = zp[32 * cg:32 * cg + 32, 512 * b_:512 * (b_ + 1)]
                    rt = []
                    for k in range(2):
                        r = rp.tile([P, N], BF16, tag="rt%d" % k)
                        if jl % 5 == 4:
                            nc.scalar.activation(
                                r, cT[:, k, :], AF.Relu,
                                bias=acolb[:, k, jl:jl + 1])
                        else:
                            nc.vector.tensor_scalar(
                                r, cT[:, k, :], acolb[:, k, jl:jl + 1], 0.0,
                                ALU.add, ALU.max)
                        rt.append(r)
                    nc.tensor.matmul(outap, w2ext_t[:, 0, :], rt[0],
                                     start=True, stop=False,
                                     tile_position=(0, 32 * cg))
                    nc.tensor.matmul(outap, w2ext_t[:, 1, :], rt[1],
                                     start=False, stop=False,
                                     tile_position=(0, 32 * cg))
                    nc.tensor.matmul(outap, eyep_t[:, jl:jl + 32], pen_t,
                                     start=False, stop=True,
                                     tile_position=(0, 32 * cg))
            nc.scalar.activation(E[:, g * 2048:(g + 1) * 2048], zp, AF.Exp,
                                 bias=fb2_bc[:, 0:1])
        rp.release()
        zpp.release()

        # reduce E over the shard's 64 j-rows (rows 0,32,64,96 of each chunk)
        fpp = tc.alloc_tile_pool(name="fpp", bufs=1, space="PSUM")
        ps_part = fpp.tile([1, N], F32)
        for g in range(4):
            for b_ in range(4):
                nc.tensor.matmul(
                    ps_part, selv_t,
                    E[:, g * 2048 + b_ * 512:g * 2048 + (b_ + 1) * 512],
                    start=(g == 0 and b_ == 0), stop=(g == 3 and b_ == 3))
        part_sb = ap_.tile([1, N], F32)
        nc.vector.tensor_copy(part_sb, ps_part)
        nc.sync.dma_start(out=partial, in_=part_sb)
        fpp.release()

        dp.release()
        ap_.release()
        wbig.release()
        wp.release()

    nc.compile()
    return nc


def _host_inputs(inputs):
    g = {k: np.asarray(v) for k, v in inputs.items()}
    x = _f32(g["seq_output"])
    ids_o = np.asarray(g["ids_o"]).reshape(-1)
    ids_c = np.asarray(g["ids_c"]).reshape(-1)
    mask = (ids_c[:, None] == ids_o[None, :]) & ~np.eye(N, dtype=bool)
    penT = np.where(mask.T, np.float32(PEN), np.float32(0.0))

    f_w2 = _f32(g["f_w2"])
    w2e = np.zeros((HD, 32), np.float32)
    w2e[:, 0:1] = f_w2
    sel_v = np.zeros((P, 1), np.float32)
    sel_v[0::32] = 1.0
    eyep = np.zeros((JSH, JSH + 32), np.float32)
    eyep[:, :JSH] = np.eye(JSH)

    w_pv, w_pa, w_pd = _f32(g["pv_w"]), _f32(g["pa_w"]), _f32(g["pd_w"])
    w_proj = np.concatenate([_f32(g["v_w"]) @ w_pv, _f32(g["a_w"]) @ w_pa,
                             _f32(g["d_w"]) @ w_pd,
                             np.zeros((HD, 1), np.float32)], axis=1)
    b_proj = np.concatenate([
        _f32(g["v_b"]) @ w_pv + _f32(g["pv_b"]),
        _f32(g["a_b"]) @ w_pa + _f32(g["pa_b"]),
        _f32(g["d_b"]) @ w_pd + _f32(g["pd_b"]),
        np.zeros(1, np.float32)])[None, :]

    shared = {
        "w_ha1": _f32(g["ha_w1"]), "w_ha2": _f32(g["ha_w2"]),
        "w_hc1": _f32(g["hc_w1"]), "w_hc2": _f32(g["hc_w2"]),
        "w_de1": _f32(g["de_w1"]), "w_de2": _f32(g["de_w2"]),
        "w_v": _f32(g["v_w"]), "w_a": _f32(g["a_w"]), "w_d": _f32(g["d_w"]),
        "w_proj": _f32(w_proj), "b_proj": _f32(b_proj),
        "fw1": _bf16(g["f_w1"]), "w2ext": _bf16(w2e), "selv": _bf16(sel_v),
        "b_ha1": _f32(g["ha_b1"])[None, :], "b_ha2": _f32(g["ha_b2"])[None, :],
        "b_hc1": _f32(g["hc_b1"])[None, :], "b_hc2": _f32(g["hc_b2"])[None, :],
        "b_de1": _f32(g["de_b1"])[None, :], "b_de2": _f32(g["de_b2"])[None, :],
        "b_v": _f32(g["v_b"])[None, :], "b_a": _f32(g["a_b"])[None, :],
        "b_d": _f32(g["d_b"])[None, :],
        "fb1": _f32(g["f_b1"])[:, None], "fb2": _f32(g["f_b2"])[None, :],
        "ones_r": np.ones((1, P), np.float32),
        "g_ha": _f32(g["ha_g"])[None, :], "t_ha": _f32(g["ha_bt"])[None, :],
        "g_hc": _f32(g["hc_g"])[None, :], "t_hc": _f32(g["hc_bt"])[None, :],
        "g_de": _f32(g["de_g"])[None, :],
        "eyepad": _bf16(eyep),
    }

    vad_o, vad_c = _f32(g["vad_o"]), _f32(g["vad_c"])
    de_bt = _f32(g["de_bt"])
    in_maps = []
    for c in range(NCORES):
        jlo = c * JSH
        selm = np.zeros((N, JSH), np.float32)
        selm[np.arange(jlo, jlo + JSH), np.arange(JSH)] = 1.0
        m = dict(shared)
        m["xT"] = _f32(x[c].T)
        m["xmbt"] = _f32(x[c] - de_bt[None, :])
        m["vad_t"] = vad_o[c] if c < 4 else vad_c[c - 4]
        m["pen"] = _bf16(penT[jlo:jlo + JSH, :])
        m["selm"] = _bf16(selm)
        in_maps.append(m)
    return in_maps, mask


def _combine(rs, mask):
    ha = np.stack([rs[c]["ha_out"] for c in range(NCORES)])
    hc = np.stack([rs[c]["hc_out"] for c in range(NCORES)])

    sc = np.stack([rs[c]["scal"][0] for c in range(NCORES)])
    club = 0.5 * sc[:, 0].sum() / (2 * N)
    rec = sc[:, 1].sum() / (2 * B * S * H)
    vad = sc[:, 2].sum() / (2 * B * S * 3)
    orth = sc[:, 3].sum()
    t0_mean = sc[0, 4] / N

    part = np.stack([rs[c]["partial"][0] for c in range(NCORES)])
    K = (~mask).sum(axis=1).astype(np.float32)
    sumexp = K + part.sum(axis=0)
    lse = np.log(sumexp)
    ha_loss = -(t0_mean - (lse.mean() - np.log(np.float32(N))))

    losses = np.array([club, rec, orth, ha_loss, vad], np.float32)
    return losses, ha, hc


def kernel(**inputs):
    global _COMPILED
    if _COMPILED is None:
        _COMPILED = _build()
    nc = _COMPILED
    in_maps, mask = _host_inputs(inputs)
    res = bass_utils.run_bass_kernel_spmd(
        nc, in_maps, core_ids=list(range(NCORES)))
    return _combine(res.results, mask)
